# revision 1
# baseline (speedup 1.0000x reference)
"""AWQ int4 linear kernel for Trainium2 (8 NeuronCores, SPMD).

Computes: out = (x * input_scale) @ dequant(qweight, scales, zeros).T + bias

  x:           [4, 2048, 4096] f32
  qweight:     [11008, 2048]   i32  (byte values 0..255; two 4-bit codes each,
                                     high nibble first -> in-position 2j, low -> 2j+1)
  scales/zeros:[11008, 32]     f32  (per 128-wide input group)
  input_scale: [4096]          f32
  bias:        [11008]         f32
  out:         [4, 2048, 11008] f32

Sharding: 4-way over tokens x 2-way over out_features (core = r*2 + c).
Per core: M=2048 tokens, K=4096, N=5504 outs.

Per-core kernel:
  - prologue: stream x, convert to f16, PE-transpose to xsT[k, t] (SBUF-resident,
    128KB/partition), scaling by input_scale (per-partition after transpose).
  - per 256-wide out-feature block: DMA packed qweight, unpack nibbles to int32
    (DVE shift/and), dequantize per-group with fused (q - zero) * scale
    tensor_scalar (int32 -> f16), PE-transpose to W[k, o].
  - matmuls with W[k, o-128] STATIONARY and xsT[k, t-512] moving (the wide
    moving dim amortizes the per-matmul weight load), fp32 PSUM accumulation
    -> psum[o, t]; bias (per-partition) added on the PSUM->SBUF copy; PE
    transposes the output back to [t, o] before the store.
"""

import os
import sys

for _p in ("/opt/trn_rl_repo",):
    if _p not in sys.path and os.path.isdir(_p):
        sys.path.insert(0, _p)

import numpy as np

import concourse.bass as bass
import concourse.mybir as mybir
from concourse import bacc
from concourse.masks import make_identity
from concourse.tile import TileContext

F32 = mybir.dt.float32
F16 = mybir.dt.float16
I32 = mybir.dt.int32

# Full problem shape
T_FULL = 8192
K_FULL = 4096
O_FULL = 11008

# Sharding: R-way over tokens, C-way over out_features
R_SHARDS = 4
C_SHARDS = 2
N_CORES = 8
KERNEL_REV = 5   # bump on every kernel change (feeds the fingerprint tag)


def build_nc(T, K, O, OB=256, w_bufs=3, q_bufs=2, nch=8, loop_n=1):
    """Build the per-core Bass program. T tokens, K in-features, O out-features."""
    assert T % 128 == 0 and K % 256 == 0 and O % 128 == 0
    nch = min(nch, K // 128)   # keep unpacked chunks >= one 128-wide group
    KT = K // 128          # k-tiles == dequant groups (group size 128)
    TT = T // 128
    NB = K // 2            # packed bytes per out-row
    TGW = min(512, T)      # moving-operand width (tokens) per matmul
    TG = T // TGW          # token groups

    # Bacc (not raw Bass): its finalize() runs the legalization passes that
    # split multi-semaphore waits (TRN2 instructions allow at most one wait).
    nc = bacc.Bacc()

    x_d = nc.dram_tensor("x", [T, K], F32, kind="ExternalInput")
    qw_d = nc.dram_tensor("qweight", [O, NB], I32, kind="ExternalInput")
    sc_d = nc.dram_tensor("scales", [O, KT], F32, kind="ExternalInput")
    zr_d = nc.dram_tensor("zeros", [O, KT], F32, kind="ExternalInput")
    isc_d = nc.dram_tensor("input_scale", [K], F32, kind="ExternalInput")
    b_d = nc.dram_tensor("bias", [O], F32, kind="ExternalInput")
    out_d = nc.dram_tensor("out", [T, O], F32, kind="ExternalOutput")
    # shape-bearing version tag: makes each build's HLO fingerprint unique so
    # the XLA/neuron compile caches can never alias two different BIRs
    tag_d = nc.dram_tensor("bench_tag", [1, KERNEL_REV * 16 + loop_n], F32,
                           kind="ExternalInput")

    blocks = []
    o = 0
    while o < O:
        w = min(OB, O - o)
        blocks.append((o, w))
        o += w

    with TileContext(nc) as tc:
        with tc.tile_pool(name="persist", bufs=1) as persist:
            # --- persistent small tensors ---
            ident = persist.tile([128, 128], F16, tag="ident")
            make_identity(nc, ident)
            identf = persist.tile([128, 128], F32, tag="identf")
            make_identity(nc, identf)

            # per-partition columns: isc_sb[p, kt] = input_scale[kt*128+p],
            # bias_sb[p, os] = bias[os*128+p]
            isc_sb = persist.tile([128, KT], F32, tag="iscale")
            nc.sync.dma_start(out=isc_sb, in_=isc_d.rearrange("(a b) -> b a", b=128))
            tag_sb = persist.tile([1, KERNEL_REV * 16 + loop_n], F32, tag="tag")
            nc.sync.dma_start(out=tag_sb, in_=tag_d[:])
            bias_sb = persist.tile([128, O // 128], F32, tag="bias")
            nc.sync.dma_start(out=bias_sb, in_=b_d.rearrange("(a b) -> b a", b=128))

            # xsT: resident transposed/scaled activations, f16, one tile per k-tile
            xsT = [
                persist.tile([128, T], F16, tag=f"xsT{kt}", name=f"xsT{kt}")
                for kt in range(KT)
            ]

            # optional in-NEFF repeat loop for timing (slope vs loop_n=1)
            import contextlib
            loop_cm = tc.For_i(0, loop_n, 1) if loop_n > 1 else contextlib.nullcontext()
            with loop_cm:
                # --- prologue: transpose x (scoped pools) ---
                KG = 512  # x DMA chunk width along k
                with (
                    tc.tile_pool(name="xin", bufs=3) as xin_pool,
                    tc.tile_pool(name="psx", bufs=3, space="PSUM") as psx_pool,
                ):
                    for tt in range(TT):
                        for kg in range(K // KG):
                            x32 = xin_pool.tile([128, KG], F32, tag="x32")
                            nc.sync.dma_start(
                                out=x32,
                                in_=x_d[tt * 128:(tt + 1) * 128, kg * KG:(kg + 1) * KG],
                            )
                            for j in range(KG // 128):
                                kt = kg * (KG // 128) + j
                                # transpose f32 directly; the ACT copy converts
                                # to f16 and applies input_scale (per-partition)
                                psx = psx_pool.tile([128, 128], F32, tag="psx")
                                nc.tensor.transpose(
                                    psx, x32[:, j * 128:(j + 1) * 128], identf
                                )
                                nc.scalar.activation(
                                    xsT[kt][:, tt * 128:(tt + 1) * 128],
                                    psx,
                                    mybir.ActivationFunctionType.Copy,
                                    scale=isc_sb[:, kt:kt + 1],
                                )

                # --- main: per out-feature block ---
                with (
                    tc.tile_pool(name="qw", bufs=2) as qw_pool,
                    tc.tile_pool(name="qi", bufs=2) as qi_pool,
                    tc.tile_pool(name="qf", bufs=q_bufs) as q_pool,
                    tc.tile_pool(name="wblk", bufs=w_bufs) as w_pool,
                    tc.tile_pool(name="sczr", bufs=3) as sczr_pool,
                    tc.tile_pool(name="oot", bufs=3) as oot_pool,
                    tc.tile_pool(name="outsb", bufs=3) as out_pool,
                    tc.tile_pool(name="psw", bufs=2, space="PSUM") as psw_pool,
                    tc.tile_pool(name="pso", bufs=4, space="PSUM") as pso_pool,
                    tc.tile_pool(name="pst", bufs=2, space="PSUM") as pst_pool,
                ):
                    for (o0, ow) in blocks:
                        osubs = ow // 128
                        w_t = w_pool.tile([128, KT * OB], F16, tag="wblk")
                        q_tiles = []
                        for osi in range(osubs):
                            ob = o0 + osi * 128
                            q = q_pool.tile([128, K], F16, tag="qf")
                            sc_t = sczr_pool.tile([128, KT], F32, tag="sc")
                            zr_t = sczr_pool.tile([128, KT], F32, tag="zr")
                            nc.sync.dma_start(out=sc_t, in_=sc_d[ob:ob + 128, :])
                            nc.sync.dma_start(out=zr_t, in_=zr_d[ob:ob + 128, :])
                            KC = K // nch  # unpacked k per packed-row chunk
                            for h in range(nch):
                                qw_t = qw_pool.tile([128, NB // nch], I32, tag="qw")
                                nc.sync.dma_start(
                                    out=qw_t,
                                    in_=qw_d[ob:ob + 128,
                                             h * (NB // nch):(h + 1) * (NB // nch)],
                                )
                                # unpack to int32 (bit ops can't cast dtypes):
                                # high nibble -> even k, low nibble -> odd k
                                qi = qi_pool.tile([128, KC], I32, tag="qi")
                                nc.vector.tensor_scalar(
                                    qi[:, ::2], qw_t, 4, None,
                                    op0=mybir.AluOpType.logical_shift_right,
                                )
                                nc.vector.tensor_scalar(
                                    qi[:, 1::2], qw_t, 15, None,
                                    op0=mybir.AluOpType.bitwise_and,
                                )
                                # per-group dequant: (q - zero) * scale, i32 -> f16
                                for j in range(KC // 128):
                                    g = (h * KC) // 128 + j
                                    nc.vector.tensor_scalar(
                                        q[:, g * 128:(g + 1) * 128],
                                        qi[:, j * 128:(j + 1) * 128],
                                        zr_t[:, g:g + 1], sc_t[:, g:g + 1],
                                        op0=mybir.AluOpType.subtract,
                                        op1=mybir.AluOpType.mult,
                                    )
                            q_tiles.append(q)
                        # transpose [o, k] -> [k, o] into the W block
                        for kt in range(KT):
                            psw = psw_pool.tile([128, OB], F16, tag="psw")
                            for osi in range(osubs):
                                nc.tensor.transpose(
                                    psw[:, osi * 128:(osi + 1) * 128],
                                    q_tiles[osi][:, kt * 128:(kt + 1) * 128],
                                    ident,
                                )
                            nc.scalar.activation(
                                w_t[:, kt * OB:kt * OB + ow],
                                psw[:, :ow],
                                mybir.ActivationFunctionType.Copy,
                            )

                        # matmuls: psum[o, t] += W[k, o-128].T @ xsT[k, t-512];
                        # the W tile is stationary across the TG token groups.
                        for osi in range(osubs):
                            os_glob = o0 // 128 + osi
                            ps = [
                                pso_pool.tile([128, TGW], F32, tag="pso",
                                              name=f"pso{o0}_{osi}_{tg}")
                                for tg in range(TG)
                            ]
                            for kt in range(KT):
                                lhsT = w_t[:, kt * OB + osi * 128:
                                           kt * OB + (osi + 1) * 128]
                                for tg in range(TG):
                                    nc.tensor.matmul(
                                        ps[tg],
                                        lhsT,
                                        xsT[kt][:, tg * TGW:(tg + 1) * TGW],
                                        start=(kt == 0),
                                        stop=(kt == KT - 1),
                                    )
                            for tg in range(TG):
                                # bias add (per-partition = out feature) on the
                                # PSUM->SBUF copy, still [o, t]
                                oot = oot_pool.tile([128, TGW], F32, tag="oot")
                                nc.vector.tensor_scalar(
                                    oot, ps[tg],
                                    bias_sb[:, os_glob:os_glob + 1], None,
                                    op0=mybir.AluOpType.add,
                                )
                                # transpose back to [t, o] and store
                                for j in range(TGW // 128):
                                    pst = pst_pool.tile([128, 128], F32, tag="pst")
                                    nc.tensor.transpose(
                                        pst, oot[:, j * 128:(j + 1) * 128], identf
                                    )
                                    osb = out_pool.tile([128, 128], F32, tag="outsb")
                                    nc.scalar.activation(
                                        osb, pst, mybir.ActivationFunctionType.Copy
                                    )
                                    t0 = tg * TGW + j * 128
                                    nc.sync.dma_start(
                                        out=out_d[t0:t0 + 128,
                                                  os_glob * 128:(os_glob + 1) * 128],
                                        in_=osb,
                                    )
    nc.finalize()
    return nc


_CACHED = {}


def _get_nc(T, K, O):
    key = (T, K, O)
    if key not in _CACHED:
        _CACHED[key] = build_nc(T, K, O)
    return _CACHED[key]


LAST_RESULT = {}


def make_in_maps(x, qweight, scales, zeros, input_scale, bias):
    """Shard the full inputs into per-core input maps."""
    x = np.ascontiguousarray(np.asarray(x, dtype=np.float32)).reshape(T_FULL, K_FULL)
    qweight = np.ascontiguousarray(np.asarray(qweight, dtype=np.int32))
    scales = np.ascontiguousarray(np.asarray(scales, dtype=np.float32))
    zeros = np.ascontiguousarray(np.asarray(zeros, dtype=np.float32))
    input_scale = np.ascontiguousarray(np.asarray(input_scale, dtype=np.float32))
    bias = np.ascontiguousarray(np.asarray(bias, dtype=np.float32))

    T = T_FULL // R_SHARDS
    O = O_FULL // C_SHARDS
    in_maps = []
    for core in range(N_CORES):
        r, c = core // C_SHARDS, core % C_SHARDS
        in_maps.append({
            "x": x[r * T:(r + 1) * T],
            "qweight": qweight[c * O:(c + 1) * O],
            "scales": scales[c * O:(c + 1) * O],
            "zeros": zeros[c * O:(c + 1) * O],
            "input_scale": input_scale,
            "bias": bias[c * O:(c + 1) * O],
            "bench_tag": np.zeros((1, KERNEL_REV * 16 + 1), dtype=np.float32),
        })
    return in_maps


def kernel(x, qweight, scales, zeros, input_scale, bias):
    from concourse.bass_utils import run_bass_kernel_spmd

    T = T_FULL // R_SHARDS
    O = O_FULL // C_SHARDS

    nc = _get_nc(T, K_FULL, O)
    in_maps = make_in_maps(x, qweight, scales, zeros, input_scale, bias)

    res = run_bass_kernel_spmd(
        nc, in_maps, list(range(N_CORES)),
        trace=bool(os.environ.get("AWQ_TRACE")),
    )
    LAST_RESULT["exec_time_ns"] = res.exec_time_ns
    LAST_RESULT["profile_json"] = res.profile_json

    out = np.empty((T_FULL, O_FULL), dtype=np.float32)
    for core in range(N_CORES):
        r, c = core // C_SHARDS, core % C_SHARDS
        out[r * T:(r + 1) * T, c * O:(c + 1) * O] = res.results[core]["out"]
    return out.reshape(4, 2048, O_FULL)

